# revision 11
# baseline (speedup 1.0000x reference)
"""2-layer GCN + classifier on 8 Trainium2 NeuronCores — v8.

v7 + critical-path work (measured via ntff profile of v7):
- Epilogue moved off the Scalar engine: ELU is materialized as
  h = relu(a+b) + (min(exp(a+b),1) - 1) with the relu/min/add on the
  (previously idle) Vector engine; only Exp/Ln remain on Scalar, which
  all live in ONE activation-table set (natural_log_exp_and_others) --
  v7 paid 194 x 1.5us in ACT_TABLE_LOAD thrash.
- log_softmax Ln is batched per tile-group instead of per tile.
- Materializing ELU(h) halves the second-layer weight matmuls
  (2 instead of 4 rel/neg pairs).
- T_full uses a quarter-major row layout; the AllGather is split into
  4 chunked collectives issued as soon as each quarter's tiles exist.
  Layer-2 collectives mostly fire DURING layer-1 aggregation, removing
  the inter-layer stall; layer-1 chunk-k gathers only wait on AG_k.
- Gather padding slots are -1 (ucode trims trailing negatives) instead
  of 0 (which gathered a garbage row).
"""
import sys

sys.path.insert(0, "/opt/trn_rl_repo")

import numpy as np
import ml_dtypes

import concourse.bacc as bacc
import concourse.tile as tile
from concourse import mybir
from concourse.bass_utils import run_bass_kernel_spmd

import os

N = 100000
E = 1600000
F_IN = 128
HID = 128
C_OUT = 40
NCORES = 8
NSH = N // NCORES          # 12500
P = 128
NT = (NSH + P - 1) // P    # 98
NSH_PAD = NT * P           # 12544
TG = 12
NTG = (NT + TG - 1) // TG  # 9

# chunked-AllGather (quarter-major T_full layout) vs single AllGather
# (core-major layout + 32768-row chunks), selectable for bisection.
CHUNKED_AG = bool(int(os.environ.get("KV8_AG", "1")))
NEW_EPI1 = bool(int(os.environ.get("KV8_EPI1", "1")))
NEW_EPI2 = bool(int(os.environ.get("KV8_EPI2", "1")))

# quarter-major T_full layout: quarter q holds rows [qrow0[q], qrow0[q+1])
# ordered (core, within-quarter).  Quarters are tile-aligned per shard.
QT = [25, 25, 25, 23]                    # tiles per quarter (sum = NT)
QSZ = [3200, 3200, 3200, NSH - 9600]     # rows per shard-quarter
QSTART = [0, 3200, 6400, 9600]           # start row within shard
NCHUNK = 4
if CHUNKED_AG:
    CHSZ = [NCORES * s for s in QSZ]     # rows of T_full per chunk
else:
    CHSZ = [32768, 32768, 32768, N - 3 * 32768]
CH0 = [0]
for s in CHSZ:
    CH0.append(CH0[-1] + s)              # chunk row offsets in T_full
NRUN = NTG * NCHUNK

F32 = mybir.dt.float32
BF16 = mybir.dt.bfloat16
I16 = mybir.dt.int16
AXX = mybir.AxisListType.X
ALU = mybir.AluOpType


def _split_hi_lo(w):
    hi = w.astype(ml_dtypes.bfloat16)
    lo = (w - hi.astype(np.float32)).astype(ml_dtypes.bfloat16)
    return hi, lo


def _prep_host(x, edge_index, W0, b0, W1, b1, Wl, bl):
    src = np.asarray(edge_index[0]).astype(np.int64)
    dst = np.asarray(edge_index[1]).astype(np.int64)
    loop = np.arange(N, dtype=np.int64)
    deg = np.bincount(np.concatenate([dst, loop]), minlength=N).astype(np.float64)
    dinv = 1.0 / np.sqrt(deg)
    wnorm = (dinv[src] * dinv[dst]).astype(np.float32)

    sdiag_all = np.zeros((NCORES, 128, NSH_PAD), dtype=ml_dtypes.bfloat16)
    for c in range(NCORES):
        nodes = np.arange(NSH, dtype=np.int64)
        sdiag_all[c][nodes % P, nodes] = (1.0 / deg[c * NSH + nodes]).astype(
            np.float32
        )

    # source node -> (chunk q, index within chunk)
    if CHUNKED_AG:
        # quarter-major layout
        s_core = src // NSH
        s_in = src % NSH
        s_q = np.digitize(s_in, QSTART[1:])      # 0..3
        qsz = np.asarray(QSZ)[s_q]
        qst = np.asarray(QSTART)[s_q]
        s_cidx = s_core * qsz + (s_in - qst)     # index within chunk
    else:
        # core-major layout (plain AllGather), 32768-row chunks
        s_q = np.digitize(src, CH0[1:-1])
        s_cidx = src - np.asarray(CH0)[s_q]

    core_of = dst // NSH
    per_core = []
    len_rc = np.zeros((NCORES, NRUN), dtype=np.int64)
    for c in range(NCORES):
        sel = core_of == c
        es = s_cidx[sel]
        k_id = s_q[sel]
        ed = dst[sel] - c * NSH
        w = wnorm[sel]
        t_id = ed // P
        g_id = t_id // TG
        run = g_id * NCHUNK + k_id
        key = run * NT + t_id
        order = np.argsort(key, kind="stable")
        es, ed, w, run, t_id = (es[order], ed[order], w[order],
                                run[order], t_id[order])
        len_rc[c] = np.bincount(run, minlength=NRUN)
        per_core.append((es, ed % P, w, run, t_id, k_id[order]))

    nblk_r = np.ceil(len_rc.max(axis=0) / P).astype(np.int64)   # [NRUN]
    blkstart_r = np.zeros(NRUN + 1, dtype=np.int64)
    np.cumsum(nblk_r, out=blkstart_r[1:])
    tot_blocks = int(blkstart_r[-1])
    tot_slots = tot_blocks * P

    # per-core (run, block, tile) triples -> union schedule
    MAXJ = int(nblk_r.max()) + 1
    codes_per_core = []
    pos_per_core = []
    for c in range(NCORES):
        es, slot, w, run, t_id, k_s = per_core[c]
        runstart = np.zeros(NRUN + 1, dtype=np.int64)
        np.cumsum(len_rc[c], out=runstart[1:])
        pos = np.arange(len(es), dtype=np.int64) - runstart[run]
        j = pos // P
        code = (run * MAXJ + j) * NT + t_id
        codes_per_core.append(code)
        pos_per_core.append(pos)

    union = np.unique(np.concatenate(codes_per_core))   # sorted (run, j, t)
    tot_sched = len(union)
    u_run = union // (MAXJ * NT)
    u_j = (union // NT) % MAXJ
    u_t = union % NT
    schedstart_r = np.searchsorted(u_run, np.arange(NRUN + 1))

    # stop flags: per group g, last sched entry (over k asc, entry asc) per tile
    stop_flag = np.zeros(tot_sched, dtype=bool)
    t_has = [set() for _ in range(NTG)]
    for g in range(NTG):
        last_for_t = {}
        for k in range(NCHUNK):
            r = g * NCHUNK + k
            for s in range(schedstart_r[r], schedstart_r[r + 1]):
                last_for_t[int(u_t[s])] = s
                t_has[g].add(int(u_t[s]))
        for t, s in last_for_t.items():
            stop_flag[s] = True

    sched = []   # per run: list of (j, t, ohcol, stop)
    for r in range(NRUN):
        ent = [(int(u_j[s]), int(u_t[s]), int(s), bool(stop_flag[s]))
               for s in range(schedstart_r[r], schedstart_r[r + 1])]
        sched.append(ent)

    gidx_all = np.zeros((NCORES, 128, tot_slots // 16), dtype=np.int16)
    oh_all = np.zeros((NCORES, 128, tot_sched * P), dtype=ml_dtypes.bfloat16)
    for c in range(NCORES):
        es, slot, w, run, t_id, k_s = per_core[c]
        pos = pos_per_core[c]
        idxflat = np.zeros(tot_slots, dtype=np.int16)
        idxflat[blkstart_r[run] * P + pos] = es.astype(np.int16)
        gidx_all[c] = np.tile(idxflat.reshape(-1, 16).T, (8, 1))
        ohcol = np.searchsorted(union, codes_per_core[c])
        oh_all[c][pos % P, ohcol * P + slot] = w

    xT_hi = np.zeros((NCORES, 128, NSH_PAD), dtype=ml_dtypes.bfloat16)
    xT_lo = np.zeros((NCORES, 128, NSH_PAD), dtype=ml_dtypes.bfloat16)
    for c in range(NCORES):
        xs = np.asarray(x[c * NSH : (c + 1) * NSH]).astype(np.float32).T
        hi, lo = _split_hi_lo(xs)
        xT_hi[c, :, :NSH] = hi
        xT_lo[c, :, :NSH] = lo

    W0h, W0l = _split_hi_lo(np.asarray(W0, dtype=np.float32))
    W1h, W1l = _split_hi_lo(np.asarray(W1, dtype=np.float32))
    Wlh, Wll = _split_hi_lo(np.asarray(Wl, dtype=np.float32))
    b0c = np.asarray(b0, dtype=np.float32).reshape(128, 1)
    b1c = np.asarray(b1, dtype=np.float32).reshape(128, 1)
    blrow = np.asarray(bl, dtype=np.float32).reshape(1, C_OUT).astype(
        ml_dtypes.bfloat16)

    in_maps = []
    for c in range(NCORES):
        in_maps.append(
            {
                "xT_hi": xT_hi[c], "xT_lo": xT_lo[c],
                "gidx": gidx_all[c], "oh": oh_all[c], "sdiag": sdiag_all[c],
                "W0h": W0h, "W0l": W0l,
                "W1h": W1h, "W1l": W1l,
                "Wlh": Wlh, "Wll": Wll,
                "b0c": b0c, "b1c": b1c, "blrow": blrow,
            }
        )
    meta = dict(sched=sched, nblk_r=nblk_r, blkstart_r=blkstart_r,
                schedstart_r=schedstart_r, t_has=t_has,
                tot_blocks=tot_blocks, tot_slots=tot_slots,
                tot_sched=tot_sched)
    return in_maps, meta


def _build_program(meta):
    sched = meta["sched"]
    nblk_r = meta["nblk_r"]
    blkstart_r = meta["blkstart_r"]
    schedstart_r = meta["schedstart_r"]
    t_has = meta["t_has"]
    tot_slots = meta["tot_slots"]
    tot_sched = meta["tot_sched"]
    max_blk = int(nblk_r.max())
    max_sched = int(max(schedstart_r[r + 1] - schedstart_r[r]
                        for r in range(NRUN)))

    nc = bacc.Bacc(num_devices=NCORES, num_swdge_queues=4)
    xT_hi = nc.declare_dram_parameter("xT_hi", [128, NSH_PAD], BF16, isOutput=False)
    xT_lo = nc.declare_dram_parameter("xT_lo", [128, NSH_PAD], BF16, isOutput=False)
    gidx = nc.declare_dram_parameter("gidx", [128, tot_slots // 16], I16,
                                     isOutput=False)
    ohp = nc.declare_dram_parameter("oh", [128, tot_sched * P], BF16,
                                    isOutput=False)
    sdiag = nc.declare_dram_parameter("sdiag", [128, NSH_PAD], BF16,
                                      isOutput=False)
    W0h = nc.declare_dram_parameter("W0h", [128, HID], BF16, isOutput=False)
    W0l = nc.declare_dram_parameter("W0l", [128, HID], BF16, isOutput=False)
    W1h = nc.declare_dram_parameter("W1h", [128, HID], BF16, isOutput=False)
    W1l = nc.declare_dram_parameter("W1l", [128, HID], BF16, isOutput=False)
    Wlh = nc.declare_dram_parameter("Wlh", [128, C_OUT], BF16, isOutput=False)
    Wll = nc.declare_dram_parameter("Wll", [128, C_OUT], BF16, isOutput=False)
    b0c = nc.declare_dram_parameter("b0c", [128, 1], F32, isOutput=False)
    b1c = nc.declare_dram_parameter("b1c", [128, 1], F32, isOutput=False)
    blrow = nc.declare_dram_parameter("blrow", [1, C_OUT], BF16, isOutput=False)
    out_ext = nc.declare_dram_parameter("out", [NSH, C_OUT], F32, isOutput=True)

    t1_shard = nc.dram_tensor("t1_shard", [NSH, HID], BF16)
    t2_shard = nc.dram_tensor("t2_shard", [NSH, HID], BF16)
    T1_full = nc.dram_tensor("T1_full", [N, HID], BF16, addr_space="Shared")
    T2_full = nc.dram_tensor("T2_full", [N, HID], BF16, addr_space="Shared")

    AF = mybir.ActivationFunctionType

    from contextlib import ExitStack
    with tile.TileContext(nc) as tc, ExitStack() as es:
        cpool = es.enter_context(tc.tile_pool(name="const", bufs=1))
        tpool = es.enter_context(tc.tile_pool(name="tsh", bufs=1))
        xpool = es.enter_context(tc.tile_pool(name="xp", bufs=3))
        gpool = es.enter_context(tc.tile_pool(name="gp", bufs=3))
        opool = es.enter_context(tc.tile_pool(name="ohp", bufs=2))
        dpool = es.enter_context(tc.tile_pool(name="dg", bufs=2))
        zpool = es.enter_context(tc.tile_pool(name="zp", bufs=4))
        lpool = es.enter_context(tc.tile_pool(name="lg", bufs=2))
        apsum = es.enter_context(tc.tile_pool(name="apsum", bufs=2, space="PSUM"))
        wpsum = es.enter_context(tc.tile_pool(name="wpsum", bufs=2, space="PSUM"))

        # ---- constants ----
        w0h_t = cpool.tile([128, HID], BF16, tag="w0h")
        w0l_t = cpool.tile([128, HID], BF16, tag="w0l")
        w1h_t = cpool.tile([128, HID], BF16, tag="w1h")
        w1l_t = cpool.tile([128, HID], BF16, tag="w1l")
        wlh_t = cpool.tile([128, C_OUT], BF16, tag="wlh")
        wll_t = cpool.tile([128, C_OUT], BF16, tag="wll")
        b0_t = cpool.tile([128, 1], F32, tag="b0")
        b1_t = cpool.tile([128, 1], F32, tag="b1")
        blr_t = cpool.tile([1, C_OUT], BF16, tag="blr")
        for tt, ext in [(w0h_t, W0h), (w0l_t, W0l), (w1h_t, W1h), (w1l_t, W1l),
                        (wlh_t, Wlh), (wll_t, Wll), (b0_t, b0c), (b1_t, b1c)]:
            nc.sync.dma_start(out=tt[:], in_=ext[:, :])
        nc.sync.dma_start(out=blr_t[:], in_=blrow[:, :])
        ones_t = cpool.tile([1, P], BF16, tag="ones")
        nc.vector.memset(ones_t[:], 1.0)

        # whole-program gather index stream (shared by both layers)
        gidx_t = cpool.tile([128, tot_slots // 16], I16, tag="gidx")
        nc.sync.dma_start(out=gidx_t[:], in_=gidx[:, :])

        # pinned T-shard tiles
        t1_tiles = [tpool.tile([P, HID], BF16, tag=f"t1_{t}", name=f"t1_{t}")
                    for t in range(NT)]
        t2_tiles = [tpool.tile([P, HID], BF16, tag=f"t2_{t}", name=f"t2_{t}")
                    for t in range(NT)]

        qtile0 = [0]
        for qt in QT:
            qtile0.append(qtile0[-1] + qt)

        def ag_chunk(shard, full, q, nm):
            if not CHUNKED_AG:
                return
            r0, r1 = QSTART[q], QSTART[q] + QSZ[q]
            nc.gpsimd.collective_compute(
                "AllGather", mybir.AluOpType.bypass,
                replica_groups=[list(range(NCORES))],
                ins=[shard[r0:r1, :].opt()],
                outs=[full[CH0[q]:CH0[q + 1], :].opt()],
            )

        def ag_whole(shard, full):
            if CHUNKED_AG:
                return
            nc.gpsimd.collective_compute(
                "AllGather", mybir.AluOpType.bypass,
                replica_groups=[list(range(NCORES))],
                ins=[shard[:].opt()],
                outs=[full[:].opt()],
            )

        # ---- phase 1a: own-shard T1 tiles (pinned, for self-loop diag) ----
        # quarter-chunked AllGathers fire as soon as each quarter's tiles
        # are stored.
        q_next = 0
        SL = 8
        for t0 in range(0, NT, SL):
            nt_s = min(SL, NT - t0)
            xh = xpool.tile([128, SL * P], BF16, tag="xh")
            xl = xpool.tile([128, SL * P], BF16, tag="xl")
            nc.sync.dma_start(out=xh[:, : nt_s * P],
                              in_=xT_hi[:, t0 * P : (t0 + nt_s) * P])
            nc.sync.dma_start(out=xl[:, : nt_s * P],
                              in_=xT_lo[:, t0 * P : (t0 + nt_s) * P])
            for i in range(nt_s):
                t = t0 + i
                ps = wpsum.tile([P, HID], F32, tag="wps", space="PSUM")
                nc.tensor.matmul(out=ps[:], lhsT=xh[:, i * P : (i + 1) * P],
                                 rhs=w0h_t[:], start=True, stop=False)
                nc.tensor.matmul(out=ps[:], lhsT=xh[:, i * P : (i + 1) * P],
                                 rhs=w0l_t[:], start=False, stop=False)
                nc.tensor.matmul(out=ps[:], lhsT=xl[:, i * P : (i + 1) * P],
                                 rhs=w0h_t[:], start=False, stop=True)
                tb = t1_tiles[t]
                nc.scalar.activation(tb[:], ps[:], AF.Copy)
                rows = min(P, NSH - t * P)
                nc.sync.dma_start(out=t1_shard[t * P : t * P + rows, :],
                                  in_=tb[:rows, :])
                if q_next < NCHUNK and t + 1 == qtile0[q_next + 1]:
                    ag_chunk(t1_shard, T1_full, q_next, "ag1")
                    q_next += 1
        ag_whole(t1_shard, T1_full)

        def agg_layer(gsrc, t_tiles, out_tiles, layer, t2_ready):
            bias_t = b0_t if layer == 1 else b1_t
            for g in range(NTG):
                tiles = list(range(g * TG, min((g + 1) * TG, NT)))
                nbank = (len(tiles) + 3) // 4
                banks = [apsum.tile([P, 512], F32, tag=f"agg{i}", space="PSUM",
                                    name=f"aggbank{i}")
                         for i in range(nbank)]

                def agg_ap(ti):
                    i = tiles.index(ti)
                    return banks[i // 4][:, (i % 4) * P : (i % 4 + 1) * P]

                # sdiag slab for this group (Act HWDGE queue)
                sds = dpool.tile([128, TG * P], BF16, tag="sds")
                nc.scalar.dma_start(
                    out=sds[:, : len(tiles) * P],
                    in_=sdiag[:, g * TG * P : g * TG * P + len(tiles) * P])

                # self-loop diagonal opens each tile's PSUM group
                for i, t in enumerate(tiles):
                    rows = min(P, NSH - t * P)
                    nc.tensor.matmul(out=agg_ap(t), lhsT=t_tiles[t][:rows, :],
                                     rhs=sds[:rows, i * P : (i + 1) * P],
                                     start=True, stop=(t not in t_has[g]),
                                     skip_group_check=True)

                for k in range(NCHUNK):
                    r = g * NCHUNK + k
                    nblk = int(nblk_r[r])
                    if nblk == 0:
                        continue
                    s_gk = nblk * P
                    nsched = int(schedstart_r[r + 1] - schedstart_r[r])
                    oht = opool.tile([128, max_sched * P], BF16, tag="oh")
                    nc.scalar.dma_start(
                        out=oht[:, : nsched * P],
                        in_=ohp[:, int(schedstart_r[r]) * P :
                                int(schedstart_r[r + 1]) * P])
                    gbuf = gpool.tile([P, max_blk, P], BF16, tag="gath")
                    a16 = int(blkstart_r[r]) * 8
                    nc.gpsimd.dma_gather(
                        gbuf[:, :nblk, :], gsrc(k),
                        gidx_t[:, a16 : a16 + s_gk // 16], s_gk, s_gk, HID,
                        single_packet=False, queue_num=k,
                    )
                    for (j, t, ohc, stop) in sched[r]:
                        lc = ohc - int(schedstart_r[r])
                        nc.tensor.matmul(
                            out=agg_ap(t),
                            lhsT=gbuf[:, j, :],
                            rhs=oht[:, lc * P : (lc + 1) * P],
                            start=False,
                            stop=stop,
                            skip_group_check=True,
                        )

                # ---- epilogue per tile ----
                # ELU(a) = relu(a+b) + (min(exp(a+b),1) - 1)
                if layer == 2:
                    nmxb = zpool.tile([128, TG], F32, tag="nmxb")
                    smb = zpool.tile([128, TG], F32, tag="smb")
                    lgp = lpool.tile([128, TG * C_OUT], F32, tag="lgs")
                for i, t in enumerate(tiles):
                    rows = min(P, NSH - t * P)
                    a1 = agg_ap(t)
                    # PSUM reads stay on the Act engine (DVE PSUM reads are
                    # ~10x slower); SBUF-to-SBUF combine runs on DVE.
                    e_t = zpool.tile([P, P], BF16, tag="e")
                    nc.scalar.activation(e_t[:], a1, AF.Exp, bias=bias_t[:])
                    r_t = zpool.tile([P, P], BF16, tag="r")
                    nc.scalar.activation(r_t[:], a1, AF.Relu, bias=bias_t[:])
                    m_t = zpool.tile([P, P], BF16, tag="m")
                    nc.vector.tensor_scalar(out=m_t[:], in0=e_t[:],
                                            scalar1=1.0, scalar2=-1.0,
                                            op0=ALU.min, op1=ALU.add)
                    h_t = zpool.tile([P, P], BF16, tag="h")
                    nc.vector.tensor_tensor(out=h_t[:], in0=r_t[:],
                                            in1=m_t[:], op=ALU.add)
                    if layer == 1:
                        ps2 = wpsum.tile([P, HID], F32, tag="wps", space="PSUM")
                        nc.tensor.matmul(out=ps2[:], lhsT=h_t[:], rhs=w1h_t[:],
                                         start=True, stop=False)
                        nc.tensor.matmul(out=ps2[:], lhsT=h_t[:], rhs=w1l_t[:],
                                         start=False, stop=True)
                        t2b = out_tiles[t]
                        nc.scalar.activation(t2b[:], ps2[:], AF.Copy)
                        nc.sync.dma_start(out=t2_shard[t * P : t * P + rows, :],
                                          in_=t2b[:rows, :])
                        t2_ready(t)
                    else:
                        psw = wpsum.tile([P, HID], F32, tag="wps", space="PSUM")
                        ps3 = psw[:, :C_OUT]
                        nc.tensor.matmul(out=ps3, lhsT=h_t[:], rhs=wlh_t[:],
                                         start=True, stop=False)
                        nc.tensor.matmul(out=ps3, lhsT=h_t[:], rhs=wll_t[:],
                                         start=False, stop=False)
                        nc.tensor.matmul(out=ps3, lhsT=ones_t[:], rhs=blr_t[:],
                                         start=False, stop=True)
                        lg = lgp[:, i * C_OUT : (i + 1) * C_OUT]
                        nc.scalar.activation(lg, ps3, AF.Copy)
                        mx = zpool.tile([P, 1], F32, tag="mx")
                        nc.vector.tensor_reduce(out=mx[:], in_=lg,
                                                axis=AXX, op=ALU.max)
                        nc.vector.tensor_scalar(out=nmxb[:, i : i + 1],
                                                in0=mx[:], scalar1=-1.0,
                                                scalar2=None, op0=ALU.mult)
                        exd = zpool.tile([P, C_OUT], BF16, tag="exd")
                        nc.scalar.activation(exd[:], lg, AF.Exp,
                                             bias=nmxb[:, i : i + 1],
                                             accum_out=smb[:, i : i + 1])
                if layer == 2:
                    nt_g = len(tiles)
                    lnb = zpool.tile([128, TG], F32, tag="lnb")
                    nc.scalar.activation(lnb[:, :nt_g], smb[:, :nt_g], AF.Ln)
                    nlsn = zpool.tile([128, TG], F32, tag="nlsn")
                    nc.vector.tensor_tensor(out=nlsn[:, :nt_g],
                                            in0=nmxb[:, :nt_g],
                                            in1=lnb[:, :nt_g],
                                            op=ALU.subtract)
                    for i, t in enumerate(tiles):
                        rows = min(P, NSH - t * P)
                        res = zpool.tile([P, C_OUT], F32, tag="res")
                        nc.vector.tensor_scalar(
                            out=res[:], in0=lgp[:, i * C_OUT : (i + 1) * C_OUT],
                            scalar1=nlsn[:, i : i + 1], scalar2=None,
                            op0=ALU.add)
                        nc.sync.dma_start(out=out_ext[t * P : t * P + rows, :],
                                          in_=res[:rows, :])

        # layer 1: t2 quarters AllGather as soon as their tiles are stored
        q2 = [0]

        def t2_ready(t):
            if q2[0] < NCHUNK and t + 1 == qtile0[q2[0] + 1]:
                ag_chunk(t2_shard, T2_full, q2[0], "ag2")
                q2[0] += 1

        agg_layer(lambda k: T1_full[CH0[k] : CH0[k + 1], :], t1_tiles,
                  t2_tiles, 1, t2_ready)
        ag_whole(t2_shard, T2_full)
        agg_layer(lambda k: T2_full[CH0[k] : CH0[k + 1], :], t2_tiles,
                  None, 2, None)

    nc.finalize()
    return nc


_CACHE = {}


def kernel(**inputs):
    in_maps, meta = _prep_host(
        inputs["x"], inputs["edge_index"], inputs["W0"], inputs["b0"],
        inputs["W1"], inputs["b1"], inputs["Wl"], inputs["bl"])
    key = (meta["tot_blocks"], meta["tot_sched"],
           meta["nblk_r"].tobytes(),
           str(meta["sched"]).__hash__())
    if key not in _CACHE:
        _CACHE[key] = _build_program(meta)
    nc = _CACHE[key]
    trace = bool(int(__import__("os").environ.get("KERNEL_TRACE", "0")))
    res = run_bass_kernel_spmd(nc, in_maps, list(range(NCORES)), trace=trace)
    kernel.last_results = res
    out = np.concatenate([res.results[c]["out"] for c in range(NCORES)], axis=0)
    return out.astype(np.float32)


# revision 14
# speedup vs baseline: 1.4375x; 1.4375x over previous
"""2-layer GCN + classifier on 8 Trainium2 NeuronCores — v8.

v7 + critical-path work (measured via ntff profile of v7):
- Epilogue moved off the Scalar engine: ELU is materialized as
  h = relu(a+b) + (min(exp(a+b),1) - 1) with the relu/min/add on the
  (previously idle) Vector engine; only Exp/Ln remain on Scalar, which
  all live in ONE activation-table set (natural_log_exp_and_others) --
  v7 paid 194 x 1.5us in ACT_TABLE_LOAD thrash.
- log_softmax Ln is batched per tile-group instead of per tile.
- Materializing ELU(h) halves the second-layer weight matmuls
  (2 instead of 4 rel/neg pairs).
- T_full uses a quarter-major row layout; the AllGather is split into
  4 chunked collectives issued as soon as each quarter's tiles exist.
  Layer-2 collectives mostly fire DURING layer-1 aggregation, removing
  the inter-layer stall; layer-1 chunk-k gathers only wait on AG_k.
"""
import sys

sys.path.insert(0, "/opt/trn_rl_repo")

import numpy as np
import ml_dtypes

import concourse.bacc as bacc
import concourse.tile as tile
from concourse import mybir
from concourse.bass_utils import run_bass_kernel_spmd

import os

N = 100000
E = 1600000
F_IN = 128
HID = 128
C_OUT = 40
NCORES = 8
NSH = N // NCORES          # 12500
P = 128
NT = (NSH + P - 1) // P    # 98
NSH_PAD = NT * P           # 12544
TG = 12
NTG = (NT + TG - 1) // TG  # 9

# chunked-AllGather (quarter-major T_full layout) vs single AllGather
# (core-major layout + 32768-row chunks), selectable for bisection.
CHUNKED_AG = bool(int(os.environ.get("KV8_AG", "1")))
NEW_EPI1 = bool(int(os.environ.get("KV8_EPI1", "1")))
NEW_EPI2 = bool(int(os.environ.get("KV8_EPI2", "1")))

# quarter-major T_full layout: quarter q holds rows [qrow0[q], qrow0[q+1])
# ordered (core, within-quarter).  Quarters are tile-aligned per shard.
QT = [25, 25, 25, 23]                    # tiles per quarter (sum = NT)
QSZ = [3200, 3200, 3200, NSH - 9600]     # rows per shard-quarter
QSTART = [0, 3200, 6400, 9600]           # start row within shard
NCHUNK = 4
if CHUNKED_AG:
    CHSZ = [NCORES * s for s in QSZ]     # rows of T_full per chunk
else:
    CHSZ = [32768, 32768, 32768, N - 3 * 32768]
CH0 = [0]
for s in CHSZ:
    CH0.append(CH0[-1] + s)              # chunk row offsets in T_full
NRUN = NTG * NCHUNK

F32 = mybir.dt.float32
BF16 = mybir.dt.bfloat16
I16 = mybir.dt.int16
AXX = mybir.AxisListType.X
ALU = mybir.AluOpType


def _split_hi_lo(w):
    hi = w.astype(ml_dtypes.bfloat16)
    lo = (w - hi.astype(np.float32)).astype(ml_dtypes.bfloat16)
    return hi, lo


def _prep_host(x, edge_index, W0, b0, W1, b1, Wl, bl):
    src = np.asarray(edge_index[0]).astype(np.int64)
    dst = np.asarray(edge_index[1]).astype(np.int64)
    loop = np.arange(N, dtype=np.int64)
    deg = np.bincount(np.concatenate([dst, loop]), minlength=N).astype(np.float64)
    dinv = 1.0 / np.sqrt(deg)
    wnorm = (dinv[src] * dinv[dst]).astype(np.float32)

    sdiag_all = np.zeros((NCORES, 128, NSH_PAD), dtype=ml_dtypes.bfloat16)
    for c in range(NCORES):
        nodes = np.arange(NSH, dtype=np.int64)
        sdiag_all[c][nodes % P, nodes] = (1.0 / deg[c * NSH + nodes]).astype(
            np.float32
        )

    # source node -> (chunk q, index within chunk)
    if CHUNKED_AG:
        # quarter-major layout
        s_core = src // NSH
        s_in = src % NSH
        s_q = np.digitize(s_in, QSTART[1:])      # 0..3
        qsz = np.asarray(QSZ)[s_q]
        qst = np.asarray(QSTART)[s_q]
        s_cidx = s_core * qsz + (s_in - qst)     # index within chunk
    else:
        # core-major layout (plain AllGather), 32768-row chunks
        s_q = np.digitize(src, CH0[1:-1])
        s_cidx = src - np.asarray(CH0)[s_q]

    core_of = dst // NSH
    per_core = []
    len_rc = np.zeros((NCORES, NRUN), dtype=np.int64)
    for c in range(NCORES):
        sel = core_of == c
        es = s_cidx[sel]
        k_id = s_q[sel]
        ed = dst[sel] - c * NSH
        w = wnorm[sel]
        t_id = ed // P
        g_id = t_id // TG
        run = g_id * NCHUNK + k_id
        key = run * NT + t_id
        order = np.argsort(key, kind="stable")
        es, ed, w, run, t_id = (es[order], ed[order], w[order],
                                run[order], t_id[order])
        len_rc[c] = np.bincount(run, minlength=NRUN)
        per_core.append((es, ed % P, w, run, t_id, k_id[order]))

    nblk_r = np.ceil(len_rc.max(axis=0) / P).astype(np.int64)   # [NRUN]
    blkstart_r = np.zeros(NRUN + 1, dtype=np.int64)
    np.cumsum(nblk_r, out=blkstart_r[1:])
    tot_blocks = int(blkstart_r[-1])
    tot_slots = tot_blocks * P

    # per-core (run, block, tile) triples -> union schedule
    MAXJ = int(nblk_r.max()) + 1
    codes_per_core = []
    pos_per_core = []
    for c in range(NCORES):
        es, slot, w, run, t_id, k_s = per_core[c]
        runstart = np.zeros(NRUN + 1, dtype=np.int64)
        np.cumsum(len_rc[c], out=runstart[1:])
        pos = np.arange(len(es), dtype=np.int64) - runstart[run]
        j = pos // P
        code = (run * MAXJ + j) * NT + t_id
        codes_per_core.append(code)
        pos_per_core.append(pos)

    union = np.unique(np.concatenate(codes_per_core))   # sorted (run, j, t)
    tot_sched = len(union)
    u_run = union // (MAXJ * NT)
    u_j = (union // NT) % MAXJ
    u_t = union % NT
    schedstart_r = np.searchsorted(u_run, np.arange(NRUN + 1))

    # stop flags: per group g, last sched entry (over k asc, entry asc) per tile
    stop_flag = np.zeros(tot_sched, dtype=bool)
    t_has = [set() for _ in range(NTG)]
    for g in range(NTG):
        last_for_t = {}
        for k in range(NCHUNK):
            r = g * NCHUNK + k
            for s in range(schedstart_r[r], schedstart_r[r + 1]):
                last_for_t[int(u_t[s])] = s
                t_has[g].add(int(u_t[s]))
        for t, s in last_for_t.items():
            stop_flag[s] = True

    sched = []   # per run: list of (j, t, ohcol, stop)
    for r in range(NRUN):
        ent = [(int(u_j[s]), int(u_t[s]), int(s), bool(stop_flag[s]))
               for s in range(schedstart_r[r], schedstart_r[r + 1])]
        sched.append(ent)

    gidx_all = np.zeros((NCORES, 128, tot_slots // 16), dtype=np.int16)
    oh_all = np.zeros((NCORES, 128, tot_sched * P), dtype=ml_dtypes.bfloat16)
    for c in range(NCORES):
        es, slot, w, run, t_id, k_s = per_core[c]
        pos = pos_per_core[c]
        idxflat = np.zeros(tot_slots, dtype=np.int16)
        idxflat[blkstart_r[run] * P + pos] = es.astype(np.int16)
        gidx_all[c] = np.tile(idxflat.reshape(-1, 16).T, (8, 1))
        ohcol = np.searchsorted(union, codes_per_core[c])
        oh_all[c][pos % P, ohcol * P + slot] = w

    xT_hi = np.zeros((NCORES, 128, NSH_PAD), dtype=ml_dtypes.bfloat16)
    xT_lo = np.zeros((NCORES, 128, NSH_PAD), dtype=ml_dtypes.bfloat16)
    for c in range(NCORES):
        xs = np.asarray(x[c * NSH : (c + 1) * NSH]).astype(np.float32).T
        hi, lo = _split_hi_lo(xs)
        xT_hi[c, :, :NSH] = hi
        xT_lo[c, :, :NSH] = lo

    W0h, W0l = _split_hi_lo(np.asarray(W0, dtype=np.float32))
    W1h, W1l = _split_hi_lo(np.asarray(W1, dtype=np.float32))
    Wlh, Wll = _split_hi_lo(np.asarray(Wl, dtype=np.float32))
    # h' = ELU+1 is what the epilogue materializes; the -1*colsum(W)
    # correction rides the ones-row matmul (free for the classifier).
    nW1s = (-np.asarray(W1, dtype=np.float32).sum(0, keepdims=True)).astype(
        ml_dtypes.bfloat16)
    b0c = np.asarray(b0, dtype=np.float32).reshape(128, 1)
    b1c = np.asarray(b1, dtype=np.float32).reshape(128, 1)
    blrow = (np.asarray(bl, dtype=np.float32)
             - np.asarray(Wl, dtype=np.float32).sum(0)).reshape(
        1, C_OUT).astype(ml_dtypes.bfloat16)

    in_maps = []
    for c in range(NCORES):
        in_maps.append(
            {
                "xT_hi": xT_hi[c], "xT_lo": xT_lo[c],
                "gidx": gidx_all[c], "oh": oh_all[c], "sdiag": sdiag_all[c],
                "W0h": W0h, "W0l": W0l,
                "W1h": W1h, "W1l": W1l,
                "Wlh": Wlh, "Wll": Wll,
                "b0c": b0c, "b1c": b1c, "blrow": blrow, "nW1s": nW1s,
            }
        )
    meta = dict(sched=sched, nblk_r=nblk_r, blkstart_r=blkstart_r,
                schedstart_r=schedstart_r, t_has=t_has,
                tot_blocks=tot_blocks, tot_slots=tot_slots,
                tot_sched=tot_sched)
    return in_maps, meta


def _build_program(meta):
    sched = meta["sched"]
    nblk_r = meta["nblk_r"]
    blkstart_r = meta["blkstart_r"]
    schedstart_r = meta["schedstart_r"]
    t_has = meta["t_has"]
    tot_slots = meta["tot_slots"]
    tot_sched = meta["tot_sched"]
    max_blk = int(nblk_r.max())
    max_sched = int(max(schedstart_r[r + 1] - schedstart_r[r]
                        for r in range(NRUN)))

    nc = bacc.Bacc(num_devices=NCORES, num_swdge_queues=4)
    xT_hi = nc.declare_dram_parameter("xT_hi", [128, NSH_PAD], BF16, isOutput=False)
    xT_lo = nc.declare_dram_parameter("xT_lo", [128, NSH_PAD], BF16, isOutput=False)
    gidx = nc.declare_dram_parameter("gidx", [128, tot_slots // 16], I16,
                                     isOutput=False)
    ohp = nc.declare_dram_parameter("oh", [128, tot_sched * P], BF16,
                                    isOutput=False)
    sdiag = nc.declare_dram_parameter("sdiag", [128, NSH_PAD], BF16,
                                      isOutput=False)
    W0h = nc.declare_dram_parameter("W0h", [128, HID], BF16, isOutput=False)
    W0l = nc.declare_dram_parameter("W0l", [128, HID], BF16, isOutput=False)
    W1h = nc.declare_dram_parameter("W1h", [128, HID], BF16, isOutput=False)
    W1l = nc.declare_dram_parameter("W1l", [128, HID], BF16, isOutput=False)
    Wlh = nc.declare_dram_parameter("Wlh", [128, C_OUT], BF16, isOutput=False)
    Wll = nc.declare_dram_parameter("Wll", [128, C_OUT], BF16, isOutput=False)
    b0c = nc.declare_dram_parameter("b0c", [128, 1], F32, isOutput=False)
    b1c = nc.declare_dram_parameter("b1c", [128, 1], F32, isOutput=False)
    blrow = nc.declare_dram_parameter("blrow", [1, C_OUT], BF16, isOutput=False)
    nW1s = nc.declare_dram_parameter("nW1s", [1, HID], BF16, isOutput=False)
    out_ext = nc.declare_dram_parameter("out", [NSH, C_OUT], F32, isOutput=True)

    t1_shard = nc.dram_tensor("t1_shard", [NSH, HID], BF16)
    t2_shard = nc.dram_tensor("t2_shard", [NSH, HID], BF16)
    T1_full = nc.dram_tensor("T1_full", [N, HID], BF16, addr_space="Shared")
    T2_full = nc.dram_tensor("T2_full", [N, HID], BF16, addr_space="Shared")

    AF = mybir.ActivationFunctionType

    from contextlib import ExitStack
    with tile.TileContext(nc) as tc, ExitStack() as es:
        cpool = es.enter_context(tc.tile_pool(name="const", bufs=1))
        tpool = es.enter_context(tc.tile_pool(name="tsh", bufs=1))
        xpool = es.enter_context(tc.tile_pool(name="xp", bufs=3))
        gpool = es.enter_context(tc.tile_pool(name="gp", bufs=3))
        opool = es.enter_context(tc.tile_pool(name="ohp", bufs=2))
        dpool = es.enter_context(tc.tile_pool(name="dg", bufs=2))
        zpool = es.enter_context(tc.tile_pool(name="zp", bufs=4))
        lpool = es.enter_context(tc.tile_pool(name="lg", bufs=2))
        apsum = es.enter_context(tc.tile_pool(name="apsum", bufs=2, space="PSUM"))
        wpsum = es.enter_context(tc.tile_pool(name="wpsum", bufs=2, space="PSUM"))

        # ---- constants ----
        w0h_t = cpool.tile([128, HID], BF16, tag="w0h")
        w0l_t = cpool.tile([128, HID], BF16, tag="w0l")
        w1h_t = cpool.tile([128, HID], BF16, tag="w1h")
        w1l_t = cpool.tile([128, HID], BF16, tag="w1l")
        wlh_t = cpool.tile([128, C_OUT], BF16, tag="wlh")
        wll_t = cpool.tile([128, C_OUT], BF16, tag="wll")
        b0_t = cpool.tile([128, 1], F32, tag="b0")
        b1_t = cpool.tile([128, 1], F32, tag="b1")
        blr_t = cpool.tile([1, C_OUT], BF16, tag="blr")
        n1s_t = cpool.tile([1, HID], BF16, tag="n1s")
        for tt, ext in [(w0h_t, W0h), (w0l_t, W0l), (w1h_t, W1h), (w1l_t, W1l),
                        (wlh_t, Wlh), (wll_t, Wll), (b0_t, b0c), (b1_t, b1c)]:
            nc.sync.dma_start(out=tt[:], in_=ext[:, :])
        nc.sync.dma_start(out=blr_t[:], in_=blrow[:, :])
        nc.sync.dma_start(out=n1s_t[:], in_=nW1s[:, :])
        ones_t = cpool.tile([1, P], BF16, tag="ones")
        nc.vector.memset(ones_t[:], 1.0)

        # whole-program gather index stream (shared by both layers)
        gidx_t = cpool.tile([128, tot_slots // 16], I16, tag="gidx")
        nc.sync.dma_start(out=gidx_t[:], in_=gidx[:, :])

        # pinned T-shard tiles
        t1_tiles = [tpool.tile([P, HID], BF16, tag=f"t1_{t}", name=f"t1_{t}")
                    for t in range(NT)]
        t2_tiles = [tpool.tile([P, HID], BF16, tag=f"t2_{t}", name=f"t2_{t}")
                    for t in range(NT)]

        qtile0 = [0]
        for qt in QT:
            qtile0.append(qtile0[-1] + qt)

        def ag_chunk(shard, full, q, nm):
            if not CHUNKED_AG:
                return
            r0, r1 = QSTART[q], QSTART[q] + QSZ[q]
            nc.gpsimd.collective_compute(
                "AllGather", mybir.AluOpType.bypass,
                replica_groups=[list(range(NCORES))],
                ins=[shard[r0:r1, :].opt()],
                outs=[full[CH0[q]:CH0[q + 1], :].opt()],
            )

        def ag_whole(shard, full):
            if CHUNKED_AG:
                return
            nc.gpsimd.collective_compute(
                "AllGather", mybir.AluOpType.bypass,
                replica_groups=[list(range(NCORES))],
                ins=[shard[:].opt()],
                outs=[full[:].opt()],
            )

        # ---- phase 1a: own-shard T1 tiles (pinned, for self-loop diag) ----
        # quarter-chunked AllGathers fire as soon as each quarter's tiles
        # are stored.
        q_next = 0
        SL = 8
        for t0 in range(0, NT, SL):
            nt_s = min(SL, NT - t0)
            xh = xpool.tile([128, SL * P], BF16, tag="xh")
            xl = xpool.tile([128, SL * P], BF16, tag="xl")
            nc.sync.dma_start(out=xh[:, : nt_s * P],
                              in_=xT_hi[:, t0 * P : (t0 + nt_s) * P])
            nc.sync.dma_start(out=xl[:, : nt_s * P],
                              in_=xT_lo[:, t0 * P : (t0 + nt_s) * P])
            for i in range(nt_s):
                t = t0 + i
                ps = wpsum.tile([P, HID], F32, tag="wps", space="PSUM")
                nc.tensor.matmul(out=ps[:], lhsT=xh[:, i * P : (i + 1) * P],
                                 rhs=w0h_t[:], start=True, stop=False)
                nc.tensor.matmul(out=ps[:], lhsT=xh[:, i * P : (i + 1) * P],
                                 rhs=w0l_t[:], start=False, stop=False)
                nc.tensor.matmul(out=ps[:], lhsT=xl[:, i * P : (i + 1) * P],
                                 rhs=w0h_t[:], start=False, stop=True)
                tb = t1_tiles[t]
                nc.vector.tensor_copy(out=tb[:], in_=ps[:])
                rows = min(P, NSH - t * P)
                nc.sync.dma_start(out=t1_shard[t * P : t * P + rows, :],
                                  in_=tb[:rows, :])
                if q_next < NCHUNK and t + 1 == qtile0[q_next + 1]:
                    ag_chunk(t1_shard, T1_full, q_next, "ag1")
                    q_next += 1
        ag_whole(t1_shard, T1_full)

        def agg_layer(gsrc, t_tiles, out_tiles, layer, t2_ready):
            bias_t = b0_t if layer == 1 else b1_t
            for g in range(NTG):
                tiles = list(range(g * TG, min((g + 1) * TG, NT)))
                nbank = (len(tiles) + 3) // 4
                banks = [apsum.tile([P, 512], F32, tag=f"agg{i}", space="PSUM",
                                    name=f"aggbank{i}")
                         for i in range(nbank)]

                def agg_ap(ti):
                    i = tiles.index(ti)
                    return banks[i // 4][:, (i % 4) * P : (i % 4 + 1) * P]

                # sdiag slab for this group (Act HWDGE queue)
                sds = dpool.tile([128, TG * P], BF16, tag="sds")
                nc.sync.dma_start(
                    out=sds[:, : len(tiles) * P],
                    in_=sdiag[:, g * TG * P : g * TG * P + len(tiles) * P])

                # self-loop diagonal opens each tile's PSUM group
                for i, t in enumerate(tiles):
                    rows = min(P, NSH - t * P)
                    nc.tensor.matmul(out=agg_ap(t), lhsT=t_tiles[t][:rows, :],
                                     rhs=sds[:rows, i * P : (i + 1) * P],
                                     start=True, stop=(t not in t_has[g]),
                                     skip_group_check=True)

                for k in range(NCHUNK):
                    r = g * NCHUNK + k
                    nblk = int(nblk_r[r])
                    if nblk == 0:
                        continue
                    s_gk = nblk * P
                    nsched = int(schedstart_r[r + 1] - schedstart_r[r])
                    oht = opool.tile([128, max_sched * P], BF16, tag="oh")
                    nc.sync.dma_start(
                        out=oht[:, : nsched * P],
                        in_=ohp[:, int(schedstart_r[r]) * P :
                                int(schedstart_r[r + 1]) * P])
                    gbuf = gpool.tile([P, max_blk, P], BF16, tag="gath")
                    a16 = int(blkstart_r[r]) * 8
                    nc.gpsimd.dma_gather(
                        gbuf[:, :nblk, :], gsrc(k),
                        gidx_t[:, a16 : a16 + s_gk // 16], s_gk, s_gk, HID,
                        single_packet=False, queue_num=k,
                    )
                    for (j, t, ohc, stop) in sched[r]:
                        lc = ohc - int(schedstart_r[r])
                        nc.tensor.matmul(
                            out=agg_ap(t),
                            lhsT=gbuf[:, j, :],
                            rhs=oht[:, lc * P : (lc + 1) * P],
                            start=False,
                            stop=stop,
                            skip_group_check=True,
                        )

                # ---- epilogue per tile ----
                # ELU(a) = relu(a+b) + (min(exp(a+b),1) - 1)
                if layer == 2:
                    nmxb = zpool.tile([128, TG], F32, tag="nmxb")
                    smb = zpool.tile([128, TG], F32, tag="smb")
                    lgp = lpool.tile([128, TG * C_OUT], F32, tag="lgs")
                for i, t in enumerate(tiles):
                    rows = min(P, NSH - t * P)
                    a1 = agg_ap(t)
                    # h' = ELU(a)+1 = min(exp(a+b),1) + relu(a+b)
                    e_t = zpool.tile([P, P], BF16, tag="e")
                    nc.scalar.activation(e_t[:], a1, AF.Exp, bias=bias_t[:])
                    r_t = zpool.tile([P, P], BF16, tag="r")
                    nc.vector.tensor_scalar(out=r_t[:], in0=a1,
                                            scalar1=bias_t[:], scalar2=0.0,
                                            op0=ALU.add, op1=ALU.max)
                    h_t = zpool.tile([P, P], BF16, tag="h")
                    nc.vector.scalar_tensor_tensor(out=h_t[:], in0=e_t[:],
                                                   scalar=1.0, in1=r_t[:],
                                                   op0=ALU.min, op1=ALU.add)
                    if layer == 1:
                        ps2 = wpsum.tile([P, HID], F32, tag="wps", space="PSUM")
                        nc.tensor.matmul(out=ps2[:], lhsT=h_t[:], rhs=w1h_t[:],
                                         start=True, stop=False)
                        nc.tensor.matmul(out=ps2[:], lhsT=h_t[:], rhs=w1l_t[:],
                                         start=False, stop=False)
                        nc.tensor.matmul(out=ps2[:], lhsT=ones_t[:],
                                         rhs=n1s_t[:], start=False, stop=True)
                        t2b = out_tiles[t]
                        nc.vector.tensor_copy(out=t2b[:], in_=ps2[:])
                        nc.sync.dma_start(out=t2_shard[t * P : t * P + rows, :],
                                          in_=t2b[:rows, :])
                        t2_ready(t)
                    else:
                        psw = wpsum.tile([P, HID], F32, tag="wps", space="PSUM")
                        ps3 = psw[:, :C_OUT]
                        nc.tensor.matmul(out=ps3, lhsT=h_t[:], rhs=wlh_t[:],
                                         start=True, stop=False)
                        nc.tensor.matmul(out=ps3, lhsT=h_t[:], rhs=wll_t[:],
                                         start=False, stop=False)
                        nc.tensor.matmul(out=ps3, lhsT=ones_t[:], rhs=blr_t[:],
                                         start=False, stop=True)
                        lg = lgp[:, i * C_OUT : (i + 1) * C_OUT]
                        nc.vector.tensor_copy(out=lg, in_=ps3)
                        nc.vector.tensor_reduce(out=nmxb[:, i : i + 1],
                                                in_=lg, axis=AXX, op=ALU.max,
                                                negate=True)
                        exd = zpool.tile([P, C_OUT], BF16, tag="exd")
                        nc.scalar.activation(exd[:], lg, AF.Exp,
                                             bias=nmxb[:, i : i + 1],
                                             accum_out=smb[:, i : i + 1])
                if layer == 2:
                    nt_g = len(tiles)
                    lnb = zpool.tile([128, TG], F32, tag="lnb")
                    nc.scalar.activation(lnb[:, :nt_g], smb[:, :nt_g], AF.Ln)
                    nlsn = zpool.tile([128, TG], F32, tag="nlsn")
                    nc.vector.tensor_tensor(out=nlsn[:, :nt_g],
                                            in0=nmxb[:, :nt_g],
                                            in1=lnb[:, :nt_g],
                                            op=ALU.subtract)
                    for i, t in enumerate(tiles):
                        rows = min(P, NSH - t * P)
                        res = zpool.tile([P, C_OUT], F32, tag="res")
                        nc.scalar.activation(
                            res[:], lgp[:, i * C_OUT : (i + 1) * C_OUT],
                            AF.Identity, bias=nlsn[:, i : i + 1])
                        nc.sync.dma_start(out=out_ext[t * P : t * P + rows, :],
                                          in_=res[:rows, :])

        # layer 1: t2 quarters AllGather as soon as their tiles are stored
        q2 = [0]

        def t2_ready(t):
            if q2[0] < NCHUNK and t + 1 == qtile0[q2[0] + 1]:
                ag_chunk(t2_shard, T2_full, q2[0], "ag2")
                q2[0] += 1

        agg_layer(lambda k: T1_full[CH0[k] : CH0[k + 1], :], t1_tiles,
                  t2_tiles, 1, t2_ready)
        ag_whole(t2_shard, T2_full)
        agg_layer(lambda k: T2_full[CH0[k] : CH0[k + 1], :], t2_tiles,
                  None, 2, None)

    nc.finalize()
    return nc


_CACHE = {}


def kernel(**inputs):
    in_maps, meta = _prep_host(
        inputs["x"], inputs["edge_index"], inputs["W0"], inputs["b0"],
        inputs["W1"], inputs["b1"], inputs["Wl"], inputs["bl"])
    key = (meta["tot_blocks"], meta["tot_sched"],
           meta["nblk_r"].tobytes(),
           str(meta["sched"]).__hash__())
    if key not in _CACHE:
        _CACHE[key] = _build_program(meta)
    nc = _CACHE[key]
    trace = bool(int(__import__("os").environ.get("KERNEL_TRACE", "0")))
    res = run_bass_kernel_spmd(nc, in_maps, list(range(NCORES)), trace=trace)
    kernel.last_results = res
    out = np.concatenate([res.results[c]["out"] for c in range(NCORES)], axis=0)
    return out.astype(np.float32)
